# revision 2
# baseline (speedup 1.0000x reference)
import sys
sys.path.insert(0, "/opt/trn_rl_repo")
import time
import hashlib
import numpy as np
import ml_dtypes

N_NODES = 131072
N_EDGES = 2097152
N_GRAPHS = 2048
IN_CH, HID, OUT = 12, 64, 4
NCORES = 8
NL = N_NODES // NCORES          # 16384 nodes per core
NW = NL // 128                  # 128 dst windows per core
CHUNK = 32768                   # int16 index range per table chunk
NCHUNK = N_NODES // CHUNK       # 4
GMAX = 384                      # pooled graph slots per core (3 x 128)
EW = 128                        # table row width (bf16) = 256 bytes

_prog_cache = {}


def _blob_offsets(Ttot):
    o_idx = 0
    o_dl = o_idx + 256 * Ttot          # int16 [16, 8T]
    o_x = o_dl + 128 * Ttot            # int8  [128, T]
    o_dinv = o_x + NL * IN_CH          # fp8   [128, NL*12/128]
    o_gid = o_dinv + 128 * NW * 2      # fp16  [128, NW]
    o_b2 = o_gid + 128 * NW * 2        # fp16  [128, NW]
    o_w1 = o_b2 + 128 * HID * 4        # f32   [128, HID]
    o_w2 = o_w1 + IN_CH * HID * 4      # f32   [12, 64]
    o_wfc = o_w2 + HID * HID * 4       # f32   [64, 64]
    o_b1 = o_wfc + HID * OUT * 4       # f32   [64, 4]
    return o_idx, o_dl, o_x, o_dinv, o_gid, o_b2, o_w1, o_w2, o_wfc, o_b1


def _blob_bytes(Ttot):
    return _blob_offsets(Ttot)[-1] + HID * 4   # + b1 f32 [64, 1]

LAST_EXEC_WALLS = []
LAST_EXEC_NS = []
LAST_TRACES = []


def _build(T, Tmax):
    """One fused SPMD launch over 8 cores.

    T: [NW][NCHUNK] tiles per (window, chunk) run, uniform across cores.
    Each core:
      x' shard -> AllGather -> strided x table [N, EW]
      layer 1: per window: dma_gather runs + one-hot matmul aggregation
               -> GCN conv -> p2 shard (bf16)
      AllGather p2 -> strided h table [N, EW]
      layer 2: same aggregation -> relu -> pooling one-hot matmuls
      pooled sums @ Wfc -> [GMAX, OUT]
    """
    import concourse.bass as bass
    import concourse.bacc as bacc
    import concourse.tile as tile
    import concourse.mybir as mybir
    from concourse.masks import make_identity
    dt = mybir.dt

    Ttot = int(sum(sum(r) for r in T))
    NB = _blob_bytes(Ttot)
    nc = bacc.Bacc("TRN2", target_bir_lowering=False, debug=False, num_devices=NCORES)
    blob = nc.dram_tensor("blob", [NB], dt.uint8, kind="ExternalInput").ap()
    outp = nc.dram_tensor("outp", [GMAX, OUT], dt.float32, kind="ExternalOutput").ap()

    # typed views into the blob (layout mirrored in _pack_blob)
    o_idx, o_dl, o_x, o_dinv, o_gid, o_b2, o_w1, o_w2, o_wfc, o_b1 = _blob_offsets(Ttot)
    idxin = blob[o_idx:o_dl].bitcast(dt.int16).rearrange("(p c) -> p c", p=16)
    dlin = blob[o_dl:o_x].bitcast(dt.int8).rearrange("(p c) -> p c", p=128)
    xin = blob[o_x:o_dinv].bitcast(dt.float8e4).rearrange("(p c) -> p c", p=128)
    dinv = blob[o_dinv:o_gid].bitcast(dt.float16).rearrange("(p c) -> p c", p=128)
    gid = blob[o_gid:o_b2].bitcast(dt.float16).rearrange("(p c) -> p c", p=128)
    b2 = blob[o_b2:o_w1].bitcast(dt.float32).rearrange("(p c) -> p c", p=128)
    w1 = blob[o_w1:o_w2].bitcast(dt.float32).rearrange("(p c) -> p c", p=IN_CH)
    w2 = blob[o_w2:o_wfc].bitcast(dt.float32).rearrange("(p c) -> p c", p=HID)
    wfc = blob[o_wfc:o_b1].bitcast(dt.float32).rearrange("(p c) -> p c", p=HID)
    b1 = blob[o_b1:NB].bitcast(dt.float32).rearrange("(p c) -> p c", p=HID)

    RG = [list(range(NCORES))]

    with tile.TileContext(nc) as tc:
        with tc.tile_pool(name="persist", bufs=1) as pp, \
             tc.tile_pool(name="dram", bufs=1, space="DRAM") as dp, \
             tc.tile_pool(name="g1", bufs=3) as g1p, \
             tc.tile_pool(name="g2", bufs=3) as g2p, \
             tc.tile_pool(name="oh", bufs=3) as ohp, \
             tc.tile_pool(name="work", bufs=3) as wp, \
             tc.tile_pool(name="agg", bufs=2, space="PSUM") as aggp, \
             tc.tile_pool(name="ps", bufs=1, space="PSUM") as psp, \
             tc.tile_pool(name="pacc", bufs=1, space="PSUM") as pap:
            # ---- persistent SBUF state ----
            idx_t = pp.tile([128, 8 * Ttot], dt.int16, name="idx_t")
            for k in range(8):
                nc.sync.dma_start(idx_t[16 * k:16 * (k + 1), :], idxin)
            dl8 = pp.tile([128, Ttot], dt.int8, name="dl8")
            nc.sync.dma_start(dl8[:], dlin)
            dl_f = pp.tile([128, Ttot], dt.float32, name="dl_f")
            nc.vector.tensor_copy(dl_f[:], dl8[:])
            x8 = pp.tile([128, NL * IN_CH // 128], dt.float8e4, name="x8")
            nc.sync.dma_start(x8[:], xin)
            xb = pp.tile([128, NL * IN_CH // 128], dt.bfloat16, name="xb")
            nc.vector.tensor_copy(xb[:], x8[:])
            dinv16 = pp.tile([128, NW], dt.float16, name="dinv16")
            nc.sync.dma_start(dinv16[:], dinv)
            dinv_t = pp.tile([128, NW], dt.float32, name="dinv_t")
            nc.vector.tensor_copy(dinv_t[:], dinv16[:])
            gid16 = pp.tile([128, NW], dt.float16, name="gid16")
            nc.sync.dma_start(gid16[:], gid)
            gid_t = pp.tile([128, NW], dt.float32, name="gid_t")
            nc.vector.tensor_copy(gid_t[:], gid16[:])
            w1_t = pp.tile([IN_CH, HID], dt.float32, name="w1_t")
            nc.sync.dma_start(w1_t[:], w1)
            w2_t = pp.tile([HID, HID], dt.float32, name="w2_t")
            nc.sync.dma_start(w2_t[:], w2)
            wfc_t = pp.tile([HID, OUT], dt.float32, name="wfc_t")
            nc.sync.dma_start(wfc_t[:], wfc)
            b1_t = pp.tile([HID, 1], dt.float32, name="b1_t")
            nc.sync.dma_start(b1_t[:], b1[:, 0:1])
            b2_t = pp.tile([128, HID], dt.float32, name="b2_t")
            nc.sync.dma_start(b2_t[:], b2)
            id_t = pp.tile([128, 128], dt.float32, name="id_t")
            make_identity(nc, id_t[:])
            # iota row 0..127 (bf16, for edge one-hots)
            io32 = pp.tile([128, GMAX], dt.int32, name="io32")
            nc.gpsimd.iota(io32[:], pattern=[[1, GMAX]], base=0, channel_multiplier=0)
            iof = pp.tile([128, GMAX], dt.float32, name="iof")
            nc.vector.tensor_copy(iof[:], io32[:])

            # ---- DRAM tables ----
            xq = dp.tile([NL, IN_CH], dt.bfloat16, name="xq")
            tab1s = dp.tile([N_NODES, IN_CH], dt.bfloat16, name="tab1s",
                            addr_space="Shared")
            tab1 = dp.tile([N_NODES, EW], dt.bfloat16, name="tab1")
            p2loc = dp.tile([NL, HID], dt.bfloat16, name="p2loc")
            tab2s = dp.tile([N_NODES, HID], dt.bfloat16, name="tab2s",
                            addr_space="Shared")
            tab2 = dp.tile([N_NODES, EW], dt.bfloat16, name="tab2")

            nc.sync.dma_start(
                xq[:].rearrange("(p r) d -> p (r d)", p=128), xb[:])
            nc.gpsimd.collective_compute(
                "AllGather", mybir.AluOpType.bypass, RG,
                ins=[xq.opt()], outs=[tab1s.opt()])
            for o in range(0, N_NODES, CHUNK):
                nc.sync.dma_start(tab1[o:o + CHUNK, :IN_CH],
                                  tab1s[o:o + CHUNK, :])

            # pooling accumulators live across the whole layer-2 loop
            pa = [pap.tile([128, HID], dt.float32, name=f"pa{k}")
                  for k in range(GMAX // 128)]

            # ---------------- layer 1 ----------------
            t0 = 0
            for w in range(NW):
                agg1 = aggp.tile([128, IN_CH], dt.float32, name="agg1", tag="agg1")
                wt = sum(T[w])
                first = True
                tt = t0
                for c in range(NCHUNK):
                    Twc = T[w][c]
                    if Twc == 0:
                        continue
                    G = g1p.tile([128, Tmax * EW], dt.bfloat16, name="G1", tag="G1")
                    nc.gpsimd.dma_gather(
                        out_ap=G[:, :Twc * EW].rearrange("p (t e) -> p t e", e=EW),
                        in_ap=tab1[c * CHUNK:(c + 1) * CHUNK, :],
                        idxs_ap=idx_t[:, tt * 8:(tt + Twc) * 8],
                        num_idxs=Twc * 128,
                        num_idxs_reg=Twc * 128,
                        elem_size=EW,
                    )
                    for t in range(Twc):
                        oh = ohp.tile([128, 128], dt.bfloat16, name="oh1", tag="oh1")
                        nc.vector.tensor_scalar(
                            oh[:], iof[:, :128], dl_f[:, tt + t:tt + t + 1], None,
                            op0=mybir.AluOpType.is_equal)
                        nc.tensor.matmul(
                            agg1[:], lhsT=oh[:], rhs=G[:, t * EW:t * EW + IN_CH],
                            start=first, stop=(tt + t == t0 + wt - 1),
                            skip_group_check=True)
                        first = False
                    tt += Twc
                t0 += wt
                # conv -> W1 -> relu -> W2 -> dinv -> p2 shard
                # whole matmul chain shares one PSUM bank via slices
                chain = psp.tile([128, 512], dt.float32, name="chain", tag="chain")
                o1 = wp.tile([128, IN_CH], dt.float32, name="o1", tag="o1")
                nc.scalar.mul(o1[:], agg1[:], dinv_t[:, w:w + 1])
                o1T_p = chain[:IN_CH, 0:128]
                nc.tensor.transpose(o1T_p, o1[:], id_t[:])
                o1T = wp.tile([IN_CH, 128], dt.float32, name="o1T", tag="o1T")
                nc.scalar.copy(o1T[:], o1T_p)
                h1_p = chain[:HID, 128:256]
                nc.tensor.matmul(h1_p, lhsT=w1_t[:], rhs=o1T[:],
                                 start=True, stop=True)
                h1 = wp.tile([HID, 128], dt.float32, name="h1", tag="h1")
                nc.scalar.activation(h1[:], h1_p,
                                     mybir.ActivationFunctionType.Relu,
                                     bias=b1_t[:, 0:1])
                p2_p = chain[:HID, 256:384]
                nc.tensor.matmul(p2_p, lhsT=w2_t[:], rhs=h1[:],
                                 start=True, stop=True)
                p2T = wp.tile([HID, 128], dt.float32, name="p2T", tag="p2T")
                nc.vector.tensor_copy(p2T[:], p2_p)
                p2n_p = chain[:, 384:448]
                nc.tensor.transpose(p2n_p, p2T[:], id_t[:HID, :HID])
                p2n = wp.tile([128, HID], dt.bfloat16, name="p2n", tag="p2n")
                nc.scalar.mul(p2n[:], p2n_p, dinv_t[:, w:w + 1])
                nc.sync.dma_start(p2loc[w * 128:(w + 1) * 128, :], p2n[:])

            nc.gpsimd.collective_compute(
                "AllGather", mybir.AluOpType.bypass, RG,
                ins=[p2loc.opt()], outs=[tab2s.opt()])
            for o in range(0, N_NODES, CHUNK):
                nc.sync.dma_start(tab2[o:o + CHUNK, :HID],
                                  tab2s[o:o + CHUNK, :])

            # ---------------- layer 2 + pooling ----------------
            t0 = 0
            for w in range(NW):
                agg2 = aggp.tile([128, HID], dt.float32, name="agg2", tag="agg2")
                wt = sum(T[w])
                first = True
                tt = t0
                for c in range(NCHUNK):
                    Twc = T[w][c]
                    if Twc == 0:
                        continue
                    G2 = g2p.tile([128, Tmax * EW], dt.bfloat16, name="G2", tag="G2")
                    nc.gpsimd.dma_gather(
                        out_ap=G2[:, :Twc * EW].rearrange("p (t e) -> p t e", e=EW),
                        in_ap=tab2[c * CHUNK:(c + 1) * CHUNK, :],
                        idxs_ap=idx_t[:, tt * 8:(tt + Twc) * 8],
                        num_idxs=Twc * 128,
                        num_idxs_reg=Twc * 128,
                        elem_size=EW,
                    )
                    for t in range(Twc):
                        oh = ohp.tile([128, 128], dt.bfloat16, name="oh2", tag="oh2")
                        nc.vector.tensor_scalar(
                            oh[:], iof[:, :128], dl_f[:, tt + t:tt + t + 1], None,
                            op0=mybir.AluOpType.is_equal)
                        nc.tensor.matmul(
                            agg2[:], lhsT=oh[:], rhs=G2[:, t * EW:t * EW + HID],
                            start=first, stop=(tt + t == t0 + wt - 1),
                            skip_group_check=True)
                        first = False
                    tt += Twc
                t0 += wt
                t2 = wp.tile([128, HID], dt.float32, name="t2", tag="t2")
                nc.scalar.mul(t2[:], agg2[:], dinv_t[:, w:w + 1])
                h2p = wp.tile([128, HID], dt.float32, name="h2p", tag="h2p")
                nc.vector.tensor_add(h2p[:], t2[:], b2_t[:])
                h2 = wp.tile([128, HID], dt.bfloat16, name="h2", tag="h2")
                nc.scalar.activation(h2[:], h2p[:],
                                     mybir.ActivationFunctionType.Relu)
                for k in range(GMAX // 128):
                    ohg = ohp.tile([128, 128], dt.bfloat16, name="ohg", tag="ohg")
                    nc.vector.tensor_scalar(
                        ohg[:], iof[:, k * 128:(k + 1) * 128],
                        gid_t[:, w:w + 1], None,
                        op0=mybir.AluOpType.is_equal)
                    nc.tensor.matmul(
                        pa[k][:], lhsT=ohg[:], rhs=h2[:],
                        start=(w == 0), stop=(w == NW - 1),
                        skip_group_check=True)

            # ---------------- pooled sums @ Wfc ----------------
            for k in range(GMAX // 128):
                chain = psp.tile([128, 512], dt.float32, name="chain", tag="chain")
                ps = wp.tile([128, HID], dt.float32, name="ps", tag="ps")
                nc.scalar.copy(ps[:], pa[k][:])
                psT_p = chain[:HID, 0:128]
                nc.tensor.transpose(psT_p, ps[:], id_t[:])
                psT = wp.tile([HID, 128], dt.float32, name="psT", tag="psT")
                nc.scalar.copy(psT[:], psT_p)
                o4_p = chain[:, 128:128 + OUT]
                nc.tensor.matmul(o4_p, lhsT=psT[:], rhs=wfc_t[:],
                                 start=True, stop=True)
                o4 = wp.tile([128, OUT], dt.float32, name="o4", tag="o4")
                nc.scalar.copy(o4[:], o4_p)
                nc.sync.dma_start(outp[k * 128:(k + 1) * 128, :], o4[:])

    nc.compile()
    return nc


_launcher_cache = {}


def _make_launcher(nc):
    """Persistent jitted shard_map launcher for a compiled Bass module.

    Mirrors concourse.bass2jax.run_bass_via_pjrt but keeps the jitted
    callable alive so repeat launches skip retrace/recompile.
    """
    import jax
    from jax.sharding import Mesh, PartitionSpec
    from jax.experimental.shard_map import shard_map
    from concourse import bass2jax
    import concourse.mybir as mybir

    bass2jax.install_neuronx_cc_hook()
    partition_name = nc.partition_id_tensor.name if nc.partition_id_tensor else None
    in_names, out_names, out_avals, zero_outs = [], [], [], []
    for alloc in nc.m.functions[0].allocations:
        if not isinstance(alloc, mybir.MemoryLocationSet):
            continue
        name = alloc.memorylocations[0].name
        if alloc.kind == "ExternalInput":
            if name != partition_name:
                in_names.append(name)
        elif alloc.kind == "ExternalOutput":
            shape = tuple(alloc.tensor_shape)
            dtype = mybir.dt.np(alloc.dtype)
            out_names.append(name)
            out_avals.append(jax.core.ShapedArray(shape, dtype))
            zero_outs.append((shape, dtype))
    n_params = len(in_names)
    n_outs = len(out_avals)
    full_in_names = in_names + out_names + (
        [partition_name] if partition_name else [])
    donate = tuple(range(n_params, n_params + n_outs))

    def _body(*args):
        operands = list(args)
        if partition_name is not None:
            operands.append(bass2jax.partition_id_tensor())
        outs = bass2jax._bass_exec_p.bind(
            *operands, out_avals=tuple(out_avals),
            in_names=tuple(full_in_names), out_names=tuple(out_names),
            lowering_input_output_aliases=(),
            sim_require_finite=True, sim_require_nnan=True, nc=nc)
        return tuple(outs)

    devices = jax.devices()[:NCORES]
    mesh = Mesh(np.asarray(devices), ("core",))
    sharded = jax.jit(
        shard_map(_body, mesh=mesh,
                  in_specs=(PartitionSpec("core"),) * (n_params + n_outs),
                  out_specs=(PartitionSpec("core"),) * n_outs,
                  check_rep=False),
        donate_argnums=donate, keep_unused=True)

    from jax.sharding import NamedSharding
    sharding = NamedSharding(mesh, PartitionSpec("core"))
    dev_cache = {"key": None, "arrs": None}

    def launch(in_maps, cache_key=None):
        if cache_key is not None and dev_cache["key"] == cache_key:
            dev_in = dev_cache["arrs"]
        else:
            concat_in = [
                np.concatenate([np.asarray(in_maps[c][name])
                                for c in range(NCORES)], axis=0)
                for name in in_names]
            dev_in = jax.device_put(concat_in, [sharding] * n_params)
            jax.block_until_ready(dev_in)
            if cache_key is not None:
                dev_cache["key"] = cache_key
                dev_cache["arrs"] = dev_in
        concat_zeros = [np.zeros((NCORES * s[0], *s[1:]), d)
                        for (s, d) in zero_outs]
        out_arrs = sharded(*dev_in, *concat_zeros)
        outs_np = [np.asarray(o) for o in out_arrs]
        return [
            {name: outs_np[i].reshape(NCORES, *out_avals[i].shape)[c]
             for i, name in enumerate(out_names)}
            for c in range(NCORES)]

    return launch


def _run_launch(nc, in_maps, cache_key=None):
    key = id(nc)
    if key not in _launcher_cache:
        _launcher_cache[key] = _make_launcher(nc)
    launch = _launcher_cache[key]
    t0 = time.perf_counter()
    results = launch(in_maps, cache_key=cache_key)
    LAST_EXEC_WALLS.append(time.perf_counter() - t0)
    return [r["outp"] for r in results]


def _preprocess(x, src, dst, batch, dinv):
    """Build per-core edge streams + tables. Returns dict of host arrays."""
    # edge stream: graph edges + self loops, sorted by (dst window, src)
    all_src = np.concatenate([src, np.arange(N_NODES, dtype=np.int64)])
    all_dst = np.concatenate([dst, np.arange(N_NODES, dtype=np.int64)])
    key = (all_dst >> 7) << 17 | all_src
    order = np.argsort(key, kind="stable")
    ssrc = all_src[order]
    sdst = all_dst[order]

    # run lengths per (core, window, chunk)
    grp = (sdst >> 7) * NCHUNK + (ssrc >> 15)   # 1024*4 groups, sorted order
    L = np.bincount(grp, minlength=NCORES * NW * NCHUNK).reshape(
        NCORES, NW, NCHUNK)
    Tarr = -(-L.max(axis=0) // 128)             # [NW, NCHUNK] uniform tiles
    Ttot = int(Tarr.sum())
    Tmax = int(Tarr.max())

    # per-run slot offsets in the padded stream (shared by all cores)
    run_off = np.concatenate([[0], np.cumsum(Tarr.reshape(-1) * 128)])[:-1]
    run_off = run_off.reshape(NW, NCHUNK)       # in slots

    # position of each sorted edge within its run
    gcum = np.concatenate([[0], np.cumsum(L.reshape(-1))])[:-1]
    rank_in_run = np.arange(len(ssrc)) - gcum[grp]
    d_of = (sdst >> 7) // NW
    w_of = (sdst >> 7) % NW
    c_of = (ssrc >> 15)
    pos = run_off[w_of, c_of] + rank_in_run     # slot in core d_of's stream

    NSLOT = Ttot * 128
    idx16 = np.zeros((NCORES, NSLOT), np.int16)
    dst8 = np.full((NCORES, NSLOT), -1, np.int8)
    # pad slots default: idx16=0 (chunk base row, valid), dst8=-1 (one-hot miss)
    idx16[d_of, pos] = (ssrc & (CHUNK - 1)).astype(np.int16)
    dst8[d_of, pos] = (sdst & 127).astype(np.int8)

    # wrap layouts
    j = np.arange(NSLOT)
    xp = (x * dinv[:, None]).astype(ml_dtypes.float8_e4m3)
    gfirst = [int(batch[d * NL]) for d in range(NCORES)]
    blobs = []
    for d in range(NCORES):
        ia = np.zeros((16, 8 * Ttot), np.int16)
        ia[j % 16, j // 16] = idx16[d]
        da = np.zeros((128, Ttot), np.int8)
        da[j % 128, j // 128] = dst8[d]
        dv = np.ascontiguousarray(
            dinv[d * NL:(d + 1) * NL].reshape(NW, 128).T.astype(np.float16))
        gl = (batch[d * NL:(d + 1) * NL] - gfirst[d]).astype(np.float16)
        assert gl.max() < GMAX, gl.max()
        ga = np.ascontiguousarray(gl.reshape(NW, 128).T)
        blobs.append((ia, da, np.ascontiguousarray(xp[d * NL:(d + 1) * NL]),
                      dv, ga))

    return dict(Tarr=Tarr, Ttot=Ttot, Tmax=Tmax, blobs=blobs, gfirst=gfirst)


def _pack_blob(parts, W1, b1, W2, b2, Wfc, Ttot):
    ia, da, x8, dv, ga = parts
    b2bc = np.tile(b2[None, :], (128, 1)).astype(np.float32)
    segs = [ia.tobytes(), da.tobytes(), x8.tobytes(), dv.tobytes(),
            ga.tobytes(), b2bc.tobytes(), W1.astype(np.float32).tobytes(),
            W2.astype(np.float32).tobytes(), Wfc.astype(np.float32).tobytes(),
            b1.astype(np.float32).tobytes()]
    blob = np.frombuffer(b"".join(segs), np.uint8)
    assert blob.size == _blob_bytes(Ttot), (blob.size, _blob_bytes(Ttot))
    return blob


def kernel(x, edge_index, batch, W1, b1, W2, b2, Wfc, bfc):
    x = np.asarray(x, np.float32)
    src = np.asarray(edge_index[0]).astype(np.int64)
    dst = np.asarray(edge_index[1]).astype(np.int64)
    batch = np.asarray(batch).astype(np.int64)
    W1 = np.asarray(W1, np.float32); b1 = np.asarray(b1, np.float32)
    W2 = np.asarray(W2, np.float32); b2 = np.asarray(b2, np.float32)
    Wfc = np.asarray(Wfc, np.float32); bfc = np.asarray(bfc, np.float32)

    deg = np.bincount(dst, minlength=N_NODES).astype(np.float32) + 1.0
    dinv = (1.0 / np.sqrt(deg)).astype(np.float32)

    P = _preprocess(x, src, dst, batch, dinv)

    key = hashlib.sha1(P["Tarr"].tobytes()).hexdigest()
    if key not in _prog_cache:
        _prog_cache[key] = _build(
            [[int(v) for v in row] for row in P["Tarr"]], P["Tmax"])
    nc = _prog_cache[key]

    in_maps = [
        {"blob": _pack_blob(P["blobs"][d], W1, b1[:, None], W2, b2, Wfc,
                            P["Ttot"])}
        for d in range(NCORES)]
    h = hashlib.sha1()
    for m in in_maps:
        h.update(m["blob"].tobytes())
    outs = _run_launch(nc, in_maps, cache_key=h.hexdigest())

    logits = np.zeros((N_GRAPHS, OUT), np.float32)
    for d in range(NCORES):
        gf = P["gfirst"][d]
        glast = int(batch[(d + 1) * NL - 1])
        ng = glast - gf + 1
        logits[gf:glast + 1] += outs[d][:ng]
    cnt = np.bincount(batch, minlength=N_GRAPHS).astype(np.float32)
    logits = logits / np.maximum(cnt, 1.0)[:, None] + bfc
    return (1.0 / (1.0 + np.exp(-logits))).astype(np.float32)
